# revision 81
# baseline (speedup 1.0000x reference)
"""Multi-head attention (B=4, N=2048, C=1024, H=8, Dh=128) on 8 TRN2 NeuronCores.

Sharding: head-split tensor parallel. Core c handles batch c//2 and heads
4*(c%2)..4*(c%2)+3, all 2048 queries. No device collectives: K/Q/V are
projected only for the core's own 4 heads; each core emits a partial output
projection (with half the effective output bias) and the host sums the two
partials per batch. SPMD: all cores run one graph, per-core weight slices.

Math per core (fp16 matmuls, fp32 psum):
  QKV proj, scores = Q K^T (scale folded into Wq; K-bias dropped, V-bias
  folded into b0 on host), softmax = exp(s - sampledmax - 66) via one ACT
  pass per half (per-partition bias AP + accum_out rowsum). The probability
  matrix is transposed to [key, query] layout by the DMA XBAR transpose
  (dma_start(transpose=True) on the sync HWDGE queue) instead of PE
  transposes, freeing the tensor engine for pure matmul work. PV and the
  output projection consume the transposed tiles; normalization by the
  softmax rowsum is applied to O via a DRAM-roundtrip transposed-reciprocal
  broadcast. Output y.T [1024 cout, 2048 tok] fp16 partial.
"""

import sys

if "/opt/trn_rl_repo" not in sys.path:
    sys.path.insert(0, "/opt/trn_rl_repo")

from contextlib import ExitStack

import numpy as np

import concourse.bass as bass
import concourse.mybir as mybir
from concourse import bacc
from concourse.bass_utils import run_bass_kernel_spmd
from concourse.masks import make_identity
from concourse.tile import TileContext

F32 = mybir.dt.float32
BF16 = mybir.dt.bfloat16
FP16 = mybir.dt.float16
AF = mybir.ActivationFunctionType
ALU = mybir.AluOpType

DIM = 1024
HEADS = 8
HD = 128  # head dim
B, N = 4, 2048
SCALE = float(np.sqrt(DIM / HEADS))
NCORES = 8
TOK = 2048          # query tokens per core (whole batch)
KEYS = 2048         # keys per core (whole batch)
MARGIN = 66.0       # exp bias safety margin below sampled max


def _build():
    nc = bacc.Bacc("TRN2", target_bir_lowering=False, debug=False, num_devices=NCORES)

    # head-split sharding: each core owns HL=4 heads of one batch, all 2048
    # queries; the pair's partial output projections are summed on the host.
    xT_e = nc.declare_dram_parameter("xT", [2, 8, 128, 1024], FP16, isOutput=False)
    wqT_e = nc.declare_dram_parameter("wqT", [4, 8, 128, 128], FP16, isOutput=False)
    wkT_e = nc.declare_dram_parameter("wkT", [4, 8, 128, 128], FP16, isOutput=False)
    wvT_e = nc.declare_dram_parameter("wvT", [1, 8, 128, 512], FP16, isOutput=False)
    w0T_e = nc.declare_dram_parameter("w0T", [8, 4, 128, 128], FP16, isOutput=False)
    bq_e = nc.declare_dram_parameter("bq", [128, 4], F32, isOutput=False)
    b0_e = nc.declare_dram_parameter("b0", [128, 8], F32, isOutput=False)
    out_e = nc.declare_dram_parameter("out", [DIM, TOK], FP16, isOutput=True)
    HL = 4  # local heads per core

    with TileContext(nc) as tc, ExitStack() as ctx:
        persist = ctx.enter_context(tc.tile_pool(name="persist", bufs=1))
        QT = persist.tile([128, 4, TOK], FP16)         # [d, lhead, qtok]
        KT = persist.tile([128, 4, KEYS], FP16)        # [d, lhead, key]
        V = persist.tile([128, 16, 512], BF16)         # [tok%128, keytile, lfeat]
        bq_s = persist.tile([128, 4], F32)
        b0_s = persist.tile([128, 8], F32)

        # keep the gpsimd queue free for the x loads — small/persistent
        # inputs ride the sync queue, idle until the first XBAR
        nc.sync.dma_start(out=bq_s[:, :], in_=bq_e[:, :])
        nc.sync.dma_start(out=b0_s[:, :], in_=b0_e[:, :])

        # ---------------- QK projection, two token-half phases ----------------
        # V projection is deferred: its matmuls are interleaved into the
        # first two attention groups (the attention pipeline is ACT/XBAR-
        # bound, leaving tensor-engine slack). The x tiles stay resident in
        # SBUF for it — re-reading x from DRAM serializes the DMA queue.
        wvpool = ctx.enter_context(tc.tile_pool(name="wv", bufs=1))
        wv0 = wvpool.tile([128, 8, 512], FP16)
        nc.sync.dma_start(
            out=wv0[:, :, :], in_=wvT_e[0].rearrange("c p f -> p c f"))
        xpool = ctx.enter_context(tc.tile_pool(name="xT", bufs=1))
        xts = []
        with ExitStack() as qkv_ctx:
            wp128 = qkv_ctx.enter_context(tc.tile_pool(name="w128", bufs=4))
            pq = qkv_ctx.enter_context(tc.tile_pool(name="pq", bufs=6, space="PSUM"))

            for ph in range(2):
                xt = xpool.tile([128, 8, 1024], FP16, name=f"xt{ph}",
                                tag=f"xt{ph}")
                xts.append(xt)
                if ph == 0:
                    # land the first weight tile on queue 0 before the x chunks
                    wq0 = wp128.tile([128, 8, 128], FP16, tag="w128")
                    nc.gpsimd.dma_start(out=wq0[:, :, :],
                                        in_=wqT_e[0].rearrange("c p f -> p c f"))
                # split the x loads across two DMA queues to halve the fill
                # (the scalar queue is idle until the first exp at ~95us)
                for c in range(8):
                    eng = nc.gpsimd if c % 2 == 0 else nc.scalar
                    eng.dma_start(out=xt[:, c, :], in_=xT_e[ph, c])

                # Q projection for this half's queries
                for ft in range(4):
                    if ph == 0 and ft == 0:
                        wq = wq0
                    else:
                        wq = wp128.tile([128, 8, 128], FP16, tag="w128")
                        nc.gpsimd.dma_start(
                            out=wq[:, :, :],
                            in_=wqT_e[ft].rearrange("c p f -> p c f"))
                    for tch in range(2):
                        ps = pq.tile([128, 512], F32)
                        for c in range(8):
                            nc.tensor.matmul(
                                ps[:, :], wq[:, c, :],
                                xt[:, c, tch * 512:(tch + 1) * 512],
                                start=(c == 0), stop=(c == 7))
                        nc.scalar.activation(
                            QT[:, ft, ph * 1024 + tch * 512:
                               ph * 1024 + (tch + 1) * 512], ps[:, :],
                            AF.Identity, bias=bq_s[:, ft:ft + 1])

                # K projection for this half's keys (drain on DVE, off ACT)
                for ft in range(4):
                    wk = wp128.tile([128, 8, 128], FP16, tag="w128")
                    nc.gpsimd.dma_start(
                        out=wk[:, :, :],
                        in_=wkT_e[ft].rearrange("c p f -> p c f"))
                    for tch in range(2):
                        ps = pq.tile([128, 512], F32)
                        for c in range(8):
                            nc.tensor.matmul(
                                ps[:, :], wk[:, c, :],
                                xt[:, c, tch * 512:(tch + 1) * 512],
                                start=(c == 0), stop=(c == 7))
                        nc.vector.tensor_copy(
                            KT[:, ft, ph * 1024 + tch * 512:
                               ph * 1024 + (tch + 1) * 512],
                            ps[:, :])

        # ---------------- attention (+ deferred V proj, + output proj) ------
        wpool2 = ctx.enter_context(tc.tile_pool(name="w0", bufs=1))
        ypool = ctx.enter_context(tc.tile_pool(name="y", bufs=3))
        otpool = ctx.enter_context(tc.tile_pool(name="ot", bufs=1))
        # per-512-token-chunk output tiles so the output projection can start
        # as soon as a chunk's last head is normalized
        OT_t = [otpool.tile([128, 4, 512], FP16, name=f"ott{i}", tag=f"ott{i}")
                for i in range(4)]
        # prefetch all output-projection weights up front — on the sync
        # queue, which is idle until the first XBAR; putting these on the
        # gpsimd queue would delay the x loads behind them
        w0s = []
        for ct in range(8):
            w0 = wpool2.tile([128, 4, 128], FP16, tag=f"w0{ct}", name=f"w0{ct}")
            nc.sync.dma_start(
                out=w0[:, :, :], in_=w0T_e[ct].rearrange("c p f -> p c f"))
            w0s.append(w0)
        # opool/utpool live in the outer scope: the drain-time PV flushes
        # need them after the attention context (and its psum pools) close
        opool = ctx.enter_context(tc.tile_pool(name="ov", bufs=1, space="PSUM"))
        utpool = ctx.enter_context(tc.tile_pool(name="ut", bufs=2))
        with ExitStack() as att_ctx:
            spool = att_ctx.enter_context(tc.tile_pool(name="sc", bufs=3, space="PSUM"))
            aux = att_ctx.enter_context(tc.tile_pool(name="aux", bufs=1, space="PSUM"))
            upool = att_ctx.enter_context(tc.tile_pool(name="u", bufs=2))
            small = att_ctx.enter_context(tc.tile_pool(name="sm", bufs=16))

            # deferred V-projection jobs: two per query-tile slot in groups
            # 0-1 (all 16 must be EMITTED before group 2's first PV matmul
            # to avoid a same-engine ordering deadlock). The psum drain goes
            # on ACT, which has slack while the PE chews the extra matmuls.
            def v_job(ph, tt):
                def run():
                    ps = aux.tile([128, 512], F32, name="vps", tag="aux")
                    for c in range(8):
                        nc.tensor.matmul(
                            ps[:, :], xts[ph][:, c, tt * 128:(tt + 1) * 128],
                            wv0[:, c, :], start=(c == 0), stop=(c == 7))
                    nc.scalar.copy(V[:, ph * 8 + tt, :], ps[:, :])
                return run

            vjobs = [v_job(ph, tt) for ph in range(2) for tt in range(8)]
            # spread over groups 0-2 in key-tile order; group 2's jobs are
            # emitted BEFORE that slot's PV chunk, which needs exactly the
            # tile emitted there (chunk qi reads V tiles 4qi..4qi+3)
            vsched = {(0, 0): 2, (0, 1): 2, (0, 2): 1, (0, 3): 1,
                      (1, 0): 2, (1, 1): 2, (1, 2): 1, (1, 3): 1,
                      (2, 0): 1, (2, 1): 1, (2, 2): 1, (2, 3): 1}

            # software pipeline over the 16 (head, qgroup) groups: group g
            # emits its scores/exp, with the PV matmuls of group g-2
            # interleaved 4-per-qi so the tensor engine stays fed during the
            # ACT-bound exp phase and never waits on the XBAR DMA (which
            # completes during group g-1).
            pends = []  # (h, qg, UT, rc4)

            def pv_chunk(pend, ps_o, qi):
                h, qg, UT4 = pend
                for kt in range(qi * 4, qi * 4 + 4):
                    nc.tensor.matmul(
                        ps_o[:, :], V[:, kt, h * 128:(h + 1) * 128],
                        UT4[:, :, kt, :],
                        start=(kt == 0), stop=(kt == 15))

            def pv_tail(pend, ps_o):
                # probs are pre-normalized, so O drains with a plain copy
                h, qg, UT4 = pend
                nc.vector.tensor_copy(OT_t[qg][:, h, :], ps_o[:, :])

            def flush_pv(pend):
                ps_o = opool.tile([128, 512], F32, tag="ov", name="ps_o")
                for qi in range(4):
                    pv_chunk(pend, ps_o, qi)
                pv_tail(pend, ps_o)



            gi = 0
            for h in range(HL):
                for qg in range(4):
                    # [k%128, qi, keytile, q%128]: fully contiguous — the XBAR
                    # transpose requires a contiguous destination (strided
                    # dests produce wrong output on HW)
                    UT4 = utpool.tile([128, 4, 16, 128], BF16)
                    ug = upool.tile([128, 4, KEYS], BF16)
                    ready = pends.pop(0) if len(pends) >= 2 else None
                    ps_o = None
                    if ready is not None:
                        ps_o = opool.tile([128, 512], F32, tag="ov",
                                          name="ps_o")
                    for qi in range(4):
                        qt = qg * 4 + qi
                        q_sl = QT[:, h, qt * 128:(qt + 1) * 128]

                        negb = small.tile([128, 1], F32, tag="negb")
                        ra = small.tile([128, 1], F32, tag="ra")
                        rb = small.tile([128, 1], F32, tag="rb")
                        ps_halves = []
                        for half in range(2):
                            ps = spool.tile([128, 1024], F32, tag="sc")
                            ps_halves.append(ps)
                            for kc in range(2):
                                nc.tensor.matmul(
                                    ps[:, kc * 512:(kc + 1) * 512], q_sl,
                                    KT[:, h, half * 1024 + kc * 512:
                                       half * 1024 + (kc + 1) * 512],
                                    start=True, stop=True)
                        for _ in range(vsched.get((gi, qi), 0)):
                            if vjobs:
                                vjobs.pop(0)()
                        if ready is not None:
                            pv_chunk(ready, ps_o, qi)
                        if (h, qg) == (HL - 1, 3):
                            # token-chunk 0 of the output projection rides
                            # the last group's slack: its OT tile completed
                            # at group 14's flush, the V-projection psum
                            # bank is long idle, and its output DMAs go on
                            # the idle gpsimd queue so the final XBAR on
                            # sync is not delayed
                            for k in range(2):
                                ct = qi * 2 + k
                                ps_op = aux.tile([128, 512], F32,
                                                 name="ps_op", tag="aux")
                                for dc in range(4):
                                    nc.tensor.matmul(
                                        ps_op[:, :], w0s[ct][:, dc, :],
                                        OT_t[0][:, dc, :],
                                        start=(dc == 0), stop=(dc == 3))
                                y = ypool.tile([128, 512], FP16, name="y")
                                nc.scalar.activation(
                                    y[:, :], ps_op[:, :], AF.Identity,
                                    bias=b0_s[:, ct:ct + 1])
                                nc.gpsimd.dma_start(
                                    out=out_e[ct * 128:(ct + 1) * 128,
                                              0:512],
                                    in_=y[:, :])
                        mx = small.tile([128, 1], F32, tag="mx")
                        with tc.high_priority(offset=30):
                            nc.vector.tensor_reduce(
                                mx[:, :],
                                ps_halves[0][:, :].rearrange(
                                    "p (n s) -> p n s", s=4)[:, :, 0],
                                axis=mybir.AxisListType.X, op=ALU.max)
                        # on DVE right after the mx reduce — avoids a
                        # cross-engine hop on the scores->exp latency chain
                        nc.vector.tensor_scalar(
                            negb[:, :], mx[:, :], -1.0, -MARGIN,
                            op0=ALU.mult, op1=ALU.add)
                        for half in range(2):
                            nc.scalar.activation(
                                ug[:, qi, half * 1024:(half + 1) * 1024],
                                ps_halves[half][:, :],
                                AF.Exp, bias=negb[:, :], scale=1.0,
                                accum_out=(ra if half == 0 else rb)[:, :])

                        # normalize the probs in [q, k] layout, where the
                        # reciprocal rowsum is a per-partition scalar — PV
                        # output needs no further normalization
                        nc.vector.tensor_tensor(
                            out=ra[:, :], in0=ra[:, :], in1=rb[:, :],
                            op=ALU.add)
                        rc1 = small.tile([128, 1], F32, tag="rc1")
                        nc.vector.reciprocal(rc1[:, :], ra[:, :])
                        nc.vector.tensor_scalar(
                            ug[:, qi, :], ug[:, qi, :], rc1[:, :], None,
                            op0=ALU.mult)

                    # transpose the whole group's [4x 128 q, 2048 k] probs to
                    # [k, q] tiles with one DMA XBAR op on the sync HWDGE
                    # queue — no tensor-engine or DVE work. Keep it OFF the
                    # scalar queue: a waiting DMA at the ACT queue head
                    # stalls the exp stream behind it.
                    nc.sync.dma_start(
                        out=UT4[:, :, :, :], in_=ug[:, :, :], transpose=True)
                    gi += 1

                    if ready is not None:
                        pv_tail(ready, ps_o)
                    pends.append((h, qg, UT4))

        # attention psum pools (spool/aux) are closed here, freeing 7 banks
        # for the output projection while the final two PV flushes (which
        # only need opool/utpool, kept in the outer scope) still pend.

        # ---------------- output projection ----------------
        # tch-outer with per-chunk OT tiles: chunks 0/1 are already complete
        # ((h3,qg0)/(h3,qg1) flushed two groups back), so their projection
        # runs while the last two groups' XBARs and PV flushes finish.
        with ExitStack() as op_ctx:
            pyp = op_ctx.enter_context(tc.tile_pool(name="py", bufs=5, space="PSUM"))

            def outproj_tch(tch):
                for ct in range(8):
                    ps = pyp.tile([128, 512], F32, name="yps")
                    for dc in range(4):
                        nc.tensor.matmul(
                            ps[:, :], w0s[ct][:, dc, :],
                            OT_t[tch][:, dc, :],
                            start=(dc == 0), stop=(dc == 3))
                    y = ypool.tile([128, 512], FP16, name="y")
                    nc.scalar.activation(
                        y[:, :], ps[:, :], AF.Identity,
                        bias=b0_s[:, ct:ct + 1])
                    nc.sync.dma_start(
                        out=out_e[ct * 128:(ct + 1) * 128,
                                  tch * 512:(tch + 1) * 512],
                        in_=y[:, :])

            # chunk 0 was emitted inside the last attention group
            outproj_tch(1)
            flush_pv(pends.pop(0))
            outproj_tch(2)
            flush_pv(pends.pop(0))
            outproj_tch(3)

    nc.compile()
    return nc


_NC = None


def _get_nc():
    global _NC
    if _NC is None:
        _NC = _build()
    return _NC


def _make_in_maps(x, W_qkv, b_qkv, W0, b0):
    x = np.asarray(x, dtype=np.float32)
    W_qkv = np.asarray(W_qkv, dtype=np.float32)
    b_qkv = np.asarray(b_qkv, dtype=np.float32)
    W0 = np.asarray(W0, dtype=np.float32)
    b0 = np.asarray(b0, dtype=np.float32)

    def tile_w(wT, fsz):
        # [1024 cin, F] -> [F/fsz, 8, 128, fsz] contiguous
        nf = wT.shape[1] // fsz
        return np.ascontiguousarray(
            wT.reshape(8, 128, nf, fsz).transpose(2, 0, 1, 3)
        ).astype(np.float16)

    # V-bias folds through the output projection (softmax rows sum to 1);
    # K-bias only shifts each score row uniformly, which softmax cancels.
    # Each core of a pair adds half of the effective output bias.
    b0_eff = 0.5 * (b0 + W0 @ b_qkv[2 * DIM:3 * DIM])
    b0r = np.ascontiguousarray(b0_eff.reshape(8, 128).T).astype(np.float32)

    in_maps = []
    for c in range(NCORES):
        b, g = c // 2, c % 2
        hs = slice(g * 512, (g + 1) * 512)  # this core's 4 heads (features)
        wqT = tile_w((W_qkv[0:DIM] * SCALE).T[:, hs], 128)
        wkT = tile_w(W_qkv[DIM:2 * DIM].T[:, hs], 128)
        wvT = tile_w(W_qkv[2 * DIM:3 * DIM].T[:, hs], 512)
        # w0T rows for this head group: [512 din, 1024 cout] -> [8ct, 4c, 128, 128]
        w0T = np.ascontiguousarray(
            W0.T[g * 512:(g + 1) * 512].reshape(4, 128, 8, 128)
            .transpose(2, 0, 1, 3)).astype(np.float16)
        bq = np.ascontiguousarray(
            (b_qkv[0:DIM] * SCALE)[hs].reshape(4, 128).T).astype(np.float32)
        xT = np.ascontiguousarray(
            x[b].T.reshape(8, 128, 2, 1024).transpose(2, 0, 1, 3)
        ).astype(np.float16)
        in_maps.append({
            "xT": xT, "wqT": wqT, "wkT": wkT, "wvT": wvT, "w0T": w0T,
            "bq": bq, "b0": b0r,
        })
    return in_maps


def _assemble(results):
    y = np.empty((B, N, DIM), dtype=np.float32)
    for b in range(B):
        y[b] = (results[2 * b]["out"].astype(np.float32)
                + results[2 * b + 1]["out"].astype(np.float32)).T
    return y


def kernel(x, W_qkv, b_qkv, W0, b0):
    nc = _get_nc()
    in_maps = _make_in_maps(x, W_qkv, b_qkv, W0, b0)
    res = run_bass_kernel_spmd(nc, in_maps, core_ids=list(range(NCORES)))
    return _assemble(res.results)


def kernel_traced(x, W_qkv, b_qkv, W0, b0, tmpdir=None):
    """Same as kernel() but with NTFF profiling; returns (output, BassKernelResults)."""
    nc = _get_nc()
    in_maps = _make_in_maps(x, W_qkv, b_qkv, W0, b0)
    res = run_bass_kernel_spmd(nc, in_maps, core_ids=list(range(NCORES)),
                               trace=True, trace_cores=[0], tmpdir=tmpdir)
    return _assemble(res.results), res


# revision 84
# speedup vs baseline: 1.0265x; 1.0265x over previous
"""Multi-head attention (B=4, N=2048, C=1024, H=8, Dh=128) on 8 TRN2 NeuronCores.

Sharding: head-split tensor parallel. Core c handles batch c//2 and heads
4*(c%2)..4*(c%2)+3, all 2048 queries. No device collectives: K/Q/V are
projected only for the core's own 4 heads; each core emits a partial output
projection (with half the effective output bias) and the host sums the two
partials per batch. SPMD: all cores run one graph, per-core weight slices.

Math per core (fp16 matmuls, fp32 psum):
  QKV proj, scores = Q K^T (scale folded into Wq; K-bias dropped, V-bias
  folded into b0 on host), softmax = exp(s - sampledmax - 66) via one ACT
  pass per half (per-partition bias AP + accum_out rowsum). The probability
  matrix is transposed to [key, query] layout by the DMA XBAR transpose
  (dma_start(transpose=True) on the sync HWDGE queue) instead of PE
  transposes, freeing the tensor engine for pure matmul work. PV and the
  output projection consume the transposed tiles; normalization by the
  softmax rowsum is applied to O via a DRAM-roundtrip transposed-reciprocal
  broadcast. Output y.T [1024 cout, 2048 tok] fp16 partial.
"""

import sys

if "/opt/trn_rl_repo" not in sys.path:
    sys.path.insert(0, "/opt/trn_rl_repo")

from contextlib import ExitStack

import numpy as np

import concourse.bass as bass
import concourse.mybir as mybir
from concourse import bacc
from concourse.bass_utils import run_bass_kernel_spmd
from concourse.masks import make_identity
from concourse.tile import TileContext

F32 = mybir.dt.float32
BF16 = mybir.dt.bfloat16
FP16 = mybir.dt.float16
AF = mybir.ActivationFunctionType
ALU = mybir.AluOpType

DIM = 1024
HEADS = 8
HD = 128  # head dim
B, N = 4, 2048
SCALE = float(np.sqrt(DIM / HEADS))
NCORES = 8
TOK = 2048          # query tokens per core (whole batch)
KEYS = 2048         # keys per core (whole batch)
MARGIN = 66.0       # exp bias safety margin below sampled max


def _build():
    nc = bacc.Bacc("TRN2", target_bir_lowering=False, debug=False, num_devices=NCORES)

    # head-split sharding: each core owns HL=4 heads of one batch, all 2048
    # queries; the pair's partial output projections are summed on the host.
    xT_e = nc.declare_dram_parameter("xT", [2, 8, 128, 1024], FP16, isOutput=False)
    wqT_e = nc.declare_dram_parameter("wqT", [4, 8, 128, 128], FP16, isOutput=False)
    wkT_e = nc.declare_dram_parameter("wkT", [4, 8, 128, 128], FP16, isOutput=False)
    wvT_e = nc.declare_dram_parameter("wvT", [1, 8, 128, 512], FP16, isOutput=False)
    w0T_e = nc.declare_dram_parameter("w0T", [8, 4, 128, 128], FP16, isOutput=False)
    bq_e = nc.declare_dram_parameter("bq", [128, 4], F32, isOutput=False)
    b0_e = nc.declare_dram_parameter("b0", [128, 8], F32, isOutput=False)
    out_e = nc.declare_dram_parameter("out", [DIM, TOK], FP16, isOutput=True)
    HL = 4  # local heads per core

    with TileContext(nc) as tc, ExitStack() as ctx:
        persist = ctx.enter_context(tc.tile_pool(name="persist", bufs=1))
        QT = persist.tile([128, 4, TOK], FP16)         # [d, lhead, qtok]
        KT = persist.tile([128, 4, KEYS], FP16)        # [d, lhead, key]
        V = persist.tile([128, 16, 512], BF16)         # [tok%128, keytile, lfeat]
        bq_s = persist.tile([128, 4], F32)
        b0_s = persist.tile([128, 8], F32)

        # keep the gpsimd queue free for the x loads — small/persistent
        # inputs ride the sync queue, idle until the first XBAR
        nc.sync.dma_start(out=bq_s[:, :], in_=bq_e[:, :])
        nc.sync.dma_start(out=b0_s[:, :], in_=b0_e[:, :])

        # ---------------- QK projection, two token-half phases ----------------
        # V projection is deferred: its matmuls are interleaved into the
        # first two attention groups (the attention pipeline is ACT/XBAR-
        # bound, leaving tensor-engine slack). The x tiles stay resident in
        # SBUF for it — re-reading x from DRAM serializes the DMA queue.
        wvpool = ctx.enter_context(tc.tile_pool(name="wv", bufs=1))
        wv0 = wvpool.tile([128, 8, 512], FP16)
        nc.sync.dma_start(
            out=wv0[:, :, :], in_=wvT_e[0].rearrange("c p f -> p c f"))
        xpool = ctx.enter_context(tc.tile_pool(name="xT", bufs=1))
        xts = []
        with ExitStack() as qkv_ctx:
            wp128 = qkv_ctx.enter_context(tc.tile_pool(name="w128", bufs=4))
            pq = qkv_ctx.enter_context(tc.tile_pool(name="pq", bufs=6, space="PSUM"))

            for ph in range(2):
                xt = xpool.tile([128, 8, 1024], FP16, name=f"xt{ph}",
                                tag=f"xt{ph}")
                xts.append(xt)
                if ph == 0:
                    # land the first weight tile on queue 0 before the x chunks
                    wq0 = wp128.tile([128, 8, 128], FP16, tag="w128")
                    nc.gpsimd.dma_start(out=wq0[:, :, :],
                                        in_=wqT_e[0].rearrange("c p f -> p c f"))
                # split the x loads across two DMA queues to halve the fill
                # (the scalar queue is idle until the first exp at ~95us)
                for c in range(8):
                    eng = nc.gpsimd if c % 2 == 0 else nc.scalar
                    eng.dma_start(out=xt[:, c, :], in_=xT_e[ph, c])

                # Q projection for this half's queries
                for ft in range(4):
                    if ph == 0 and ft == 0:
                        wq = wq0
                    else:
                        # weight tiles ride the scalar queue so they don't
                        # contend with the x chunks on gpsimd
                        wq = wp128.tile([128, 8, 128], FP16, tag="w128")
                        nc.scalar.dma_start(
                            out=wq[:, :, :],
                            in_=wqT_e[ft].rearrange("c p f -> p c f"))
                    for tch in range(2):
                        ps = pq.tile([128, 512], F32)
                        for c in range(8):
                            nc.tensor.matmul(
                                ps[:, :], wq[:, c, :],
                                xt[:, c, tch * 512:(tch + 1) * 512],
                                start=(c == 0), stop=(c == 7))
                        nc.scalar.activation(
                            QT[:, ft, ph * 1024 + tch * 512:
                               ph * 1024 + (tch + 1) * 512], ps[:, :],
                            AF.Identity, bias=bq_s[:, ft:ft + 1])

                # K projection for this half's keys (drain on DVE, off ACT)
                for ft in range(4):
                    wk = wp128.tile([128, 8, 128], FP16, tag="w128")
                    nc.scalar.dma_start(
                        out=wk[:, :, :],
                        in_=wkT_e[ft].rearrange("c p f -> p c f"))
                    for tch in range(2):
                        ps = pq.tile([128, 512], F32)
                        for c in range(8):
                            nc.tensor.matmul(
                                ps[:, :], wk[:, c, :],
                                xt[:, c, tch * 512:(tch + 1) * 512],
                                start=(c == 0), stop=(c == 7))
                        nc.vector.tensor_copy(
                            KT[:, ft, ph * 1024 + tch * 512:
                               ph * 1024 + (tch + 1) * 512],
                            ps[:, :])

        # ---------------- attention (+ deferred V proj, + output proj) ------
        wpool2 = ctx.enter_context(tc.tile_pool(name="w0", bufs=1))
        ypool = ctx.enter_context(tc.tile_pool(name="y", bufs=3))
        otpool = ctx.enter_context(tc.tile_pool(name="ot", bufs=1))
        # per-512-token-chunk output tiles so the output projection can start
        # as soon as a chunk's last head is normalized
        OT_t = [otpool.tile([128, 4, 512], FP16, name=f"ott{i}", tag=f"ott{i}")
                for i in range(4)]
        # prefetch all output-projection weights up front — on the sync
        # queue, which is idle until the first XBAR; putting these on the
        # gpsimd queue would delay the x loads behind them
        w0s = []
        for ct in range(8):
            w0 = wpool2.tile([128, 4, 128], FP16, tag=f"w0{ct}", name=f"w0{ct}")
            nc.sync.dma_start(
                out=w0[:, :, :], in_=w0T_e[ct].rearrange("c p f -> p c f"))
            w0s.append(w0)
        # opool/utpool live in the outer scope: the drain-time PV flushes
        # need them after the attention context (and its psum pools) close
        opool = ctx.enter_context(tc.tile_pool(name="ov", bufs=1, space="PSUM"))
        utpool = ctx.enter_context(tc.tile_pool(name="ut", bufs=2))
        with ExitStack() as att_ctx:
            spool = att_ctx.enter_context(tc.tile_pool(name="sc", bufs=3, space="PSUM"))
            aux = att_ctx.enter_context(tc.tile_pool(name="aux", bufs=1, space="PSUM"))
            upool = att_ctx.enter_context(tc.tile_pool(name="u", bufs=2))
            small = att_ctx.enter_context(tc.tile_pool(name="sm", bufs=16))

            # deferred V-projection jobs: two per query-tile slot in groups
            # 0-1 (all 16 must be EMITTED before group 2's first PV matmul
            # to avoid a same-engine ordering deadlock). The psum drain goes
            # on ACT, which has slack while the PE chews the extra matmuls.
            def v_job(ph, tt):
                def run():
                    ps = aux.tile([128, 512], F32, name="vps", tag="aux")
                    for c in range(8):
                        nc.tensor.matmul(
                            ps[:, :], xts[ph][:, c, tt * 128:(tt + 1) * 128],
                            wv0[:, c, :], start=(c == 0), stop=(c == 7))
                    nc.scalar.copy(V[:, ph * 8 + tt, :], ps[:, :])
                return run

            vjobs = [v_job(ph, tt) for ph in range(2) for tt in range(8)]
            # spread over groups 0-2 in key-tile order; group 2's jobs are
            # emitted BEFORE that slot's PV chunk, which needs exactly the
            # tile emitted there (chunk qi reads V tiles 4qi..4qi+3)
            vsched = {(0, 0): 2, (0, 1): 2, (0, 2): 1, (0, 3): 1,
                      (1, 0): 2, (1, 1): 2, (1, 2): 1, (1, 3): 1,
                      (2, 0): 1, (2, 1): 1, (2, 2): 1, (2, 3): 1}

            # software pipeline over the 16 (head, qgroup) groups: group g
            # emits its scores/exp, with the PV matmuls of group g-2
            # interleaved 4-per-qi so the tensor engine stays fed during the
            # ACT-bound exp phase and never waits on the XBAR DMA (which
            # completes during group g-1).
            pends = []  # (h, qg, UT, rc4)

            def pv_chunk(pend, ps_o, qi):
                h, qg, UT4 = pend
                for kt in range(qi * 4, qi * 4 + 4):
                    nc.tensor.matmul(
                        ps_o[:, :], V[:, kt, h * 128:(h + 1) * 128],
                        UT4[:, :, kt, :],
                        start=(kt == 0), stop=(kt == 15))

            def pv_tail(pend, ps_o):
                # probs are pre-normalized, so O drains with a plain copy
                h, qg, UT4 = pend
                nc.vector.tensor_copy(OT_t[qg][:, h, :], ps_o[:, :])

            def flush_pv(pend):
                ps_o = opool.tile([128, 512], F32, tag="ov", name="ps_o")
                for qi in range(4):
                    pv_chunk(pend, ps_o, qi)
                pv_tail(pend, ps_o)



            gi = 0
            for h in range(HL):
                for qg in range(4):
                    # [k%128, qi, keytile, q%128]: fully contiguous — the XBAR
                    # transpose requires a contiguous destination (strided
                    # dests produce wrong output on HW)
                    UT4 = utpool.tile([128, 4, 16, 128], BF16)
                    ug = upool.tile([128, 4, KEYS], BF16)
                    ready = pends.pop(0) if len(pends) >= 2 else None
                    ps_o = None
                    if ready is not None:
                        ps_o = opool.tile([128, 512], F32, tag="ov",
                                          name="ps_o")
                    for qi in range(4):
                        qt = qg * 4 + qi
                        q_sl = QT[:, h, qt * 128:(qt + 1) * 128]

                        negb = small.tile([128, 1], F32, tag="negb")
                        ra = small.tile([128, 1], F32, tag="ra")
                        rb = small.tile([128, 1], F32, tag="rb")
                        ps_halves = []
                        for half in range(2):
                            ps = spool.tile([128, 1024], F32, tag="sc")
                            ps_halves.append(ps)
                            for kc in range(2):
                                nc.tensor.matmul(
                                    ps[:, kc * 512:(kc + 1) * 512], q_sl,
                                    KT[:, h, half * 1024 + kc * 512:
                                       half * 1024 + (kc + 1) * 512],
                                    start=True, stop=True)
                        for _ in range(vsched.get((gi, qi), 0)):
                            if vjobs:
                                vjobs.pop(0)()
                        if ready is not None:
                            pv_chunk(ready, ps_o, qi)
                        mx = small.tile([128, 1], F32, tag="mx")
                        with tc.high_priority(offset=30):
                            nc.vector.tensor_reduce(
                                mx[:, :],
                                ps_halves[0][:, :].rearrange(
                                    "p (n s) -> p n s", s=4)[:, :, 0],
                                axis=mybir.AxisListType.X, op=ALU.max)
                        # on DVE right after the mx reduce — avoids a
                        # cross-engine hop on the scores->exp latency chain
                        nc.vector.tensor_scalar(
                            negb[:, :], mx[:, :], -1.0, -MARGIN,
                            op0=ALU.mult, op1=ALU.add)
                        for half in range(2):
                            nc.scalar.activation(
                                ug[:, qi, half * 1024:(half + 1) * 1024],
                                ps_halves[half][:, :],
                                AF.Exp, bias=negb[:, :], scale=1.0,
                                accum_out=(ra if half == 0 else rb)[:, :])

                        # normalize the probs in [q, k] layout, where the
                        # reciprocal rowsum is a per-partition scalar — PV
                        # output needs no further normalization
                        nc.vector.tensor_tensor(
                            out=ra[:, :], in0=ra[:, :], in1=rb[:, :],
                            op=ALU.add)
                        rc1 = small.tile([128, 1], F32, tag="rc1")
                        nc.vector.reciprocal(rc1[:, :], ra[:, :])
                        nc.vector.tensor_scalar(
                            ug[:, qi, :], ug[:, qi, :], rc1[:, :], None,
                            op0=ALU.mult)

                    # transpose the whole group's [4x 128 q, 2048 k] probs to
                    # [k, q] tiles with one DMA XBAR op on the sync HWDGE
                    # queue — no tensor-engine or DVE work. Keep it OFF the
                    # scalar queue: a waiting DMA at the ACT queue head
                    # stalls the exp stream behind it.
                    nc.sync.dma_start(
                        out=UT4[:, :, :, :], in_=ug[:, :, :], transpose=True)
                    gi += 1

                    if ready is not None:
                        pv_tail(ready, ps_o)
                    pends.append((h, qg, UT4))

        # attention psum pools (spool/aux) are closed here, freeing 7 banks
        # for the output projection while the final two PV flushes (which
        # only need opool/utpool, kept in the outer scope) still pend.

        # ---------------- output projection ----------------
        # tch-outer with per-chunk OT tiles: chunks 0/1 are already complete
        # ((h3,qg0)/(h3,qg1) flushed two groups back), so their projection
        # runs while the last two groups' XBARs and PV flushes finish.
        with ExitStack() as op_ctx:
            pyp = op_ctx.enter_context(tc.tile_pool(name="py", bufs=5, space="PSUM"))

            def outproj_tch(tch):
                for ct in range(8):
                    ps = pyp.tile([128, 512], F32, name="yps")
                    for dc in range(4):
                        nc.tensor.matmul(
                            ps[:, :], w0s[ct][:, dc, :],
                            OT_t[tch][:, dc, :],
                            start=(dc == 0), stop=(dc == 3))
                    y = ypool.tile([128, 512], FP16, name="y")
                    nc.scalar.activation(
                        y[:, :], ps[:, :], AF.Identity,
                        bias=b0_s[:, ct:ct + 1])
                    nc.sync.dma_start(
                        out=out_e[ct * 128:(ct + 1) * 128,
                                  tch * 512:(tch + 1) * 512],
                        in_=y[:, :])

            outproj_tch(0)
            outproj_tch(1)
            flush_pv(pends.pop(0))
            outproj_tch(2)
            flush_pv(pends.pop(0))
            outproj_tch(3)

    nc.compile()
    return nc


_NC = None


def _get_nc():
    global _NC
    if _NC is None:
        _NC = _build()
    return _NC


def _make_in_maps(x, W_qkv, b_qkv, W0, b0):
    x = np.asarray(x, dtype=np.float32)
    W_qkv = np.asarray(W_qkv, dtype=np.float32)
    b_qkv = np.asarray(b_qkv, dtype=np.float32)
    W0 = np.asarray(W0, dtype=np.float32)
    b0 = np.asarray(b0, dtype=np.float32)

    def tile_w(wT, fsz):
        # [1024 cin, F] -> [F/fsz, 8, 128, fsz] contiguous
        nf = wT.shape[1] // fsz
        return np.ascontiguousarray(
            wT.reshape(8, 128, nf, fsz).transpose(2, 0, 1, 3)
        ).astype(np.float16)

    # V-bias folds through the output projection (softmax rows sum to 1);
    # K-bias only shifts each score row uniformly, which softmax cancels.
    # Each core of a pair adds half of the effective output bias.
    b0_eff = 0.5 * (b0 + W0 @ b_qkv[2 * DIM:3 * DIM])
    b0r = np.ascontiguousarray(b0_eff.reshape(8, 128).T).astype(np.float32)

    in_maps = []
    for c in range(NCORES):
        b, g = c // 2, c % 2
        hs = slice(g * 512, (g + 1) * 512)  # this core's 4 heads (features)
        wqT = tile_w((W_qkv[0:DIM] * SCALE).T[:, hs], 128)
        wkT = tile_w(W_qkv[DIM:2 * DIM].T[:, hs], 128)
        wvT = tile_w(W_qkv[2 * DIM:3 * DIM].T[:, hs], 512)
        # w0T rows for this head group: [512 din, 1024 cout] -> [8ct, 4c, 128, 128]
        w0T = np.ascontiguousarray(
            W0.T[g * 512:(g + 1) * 512].reshape(4, 128, 8, 128)
            .transpose(2, 0, 1, 3)).astype(np.float16)
        bq = np.ascontiguousarray(
            (b_qkv[0:DIM] * SCALE)[hs].reshape(4, 128).T).astype(np.float32)
        xT = np.ascontiguousarray(
            x[b].T.reshape(8, 128, 2, 1024).transpose(2, 0, 1, 3)
        ).astype(np.float16)
        in_maps.append({
            "xT": xT, "wqT": wqT, "wkT": wkT, "wvT": wvT, "w0T": w0T,
            "bq": bq, "b0": b0r,
        })
    return in_maps


def _assemble(results):
    y = np.empty((B, N, DIM), dtype=np.float32)
    for b in range(B):
        y[b] = (results[2 * b]["out"].astype(np.float32)
                + results[2 * b + 1]["out"].astype(np.float32)).T
    return y


def kernel(x, W_qkv, b_qkv, W0, b0):
    nc = _get_nc()
    in_maps = _make_in_maps(x, W_qkv, b_qkv, W0, b0)
    res = run_bass_kernel_spmd(nc, in_maps, core_ids=list(range(NCORES)))
    return _assemble(res.results)


def kernel_traced(x, W_qkv, b_qkv, W0, b0, tmpdir=None):
    """Same as kernel() but with NTFF profiling; returns (output, BassKernelResults)."""
    nc = _get_nc()
    in_maps = _make_in_maps(x, W_qkv, b_qkv, W0, b0)
    res = run_bass_kernel_spmd(nc, in_maps, core_ids=list(range(NCORES)),
                               trace=True, trace_cores=[0], tmpdir=tmpdir)
    return _assemble(res.results), res


# revision 85
# speedup vs baseline: 1.1972x; 1.1663x over previous
"""Multi-head attention (B=4, N=2048, C=1024, H=8, Dh=128) on 8 TRN2 NeuronCores.

Sharding: head-split tensor parallel. Core c handles batch c//2 and heads
4*(c%2)..4*(c%2)+3, all 2048 queries. No device collectives: K/Q/V are
projected only for the core's own 4 heads; each core emits a partial output
projection (with half the effective output bias) and the host sums the two
partials per batch. SPMD: all cores run one graph, per-core weight slices.

Math per core (fp16 matmuls, fp32 psum):
  QKV proj, scores = Q K^T (scale folded into Wq; K-bias dropped, V-bias
  folded into b0 on host), softmax = exp(s - sampledmax - 66) via one ACT
  pass per half (per-partition bias AP + accum_out rowsum). The probability
  matrix is transposed to [key, query] layout by the DMA XBAR transpose
  (dma_start(transpose=True) on the sync HWDGE queue) instead of PE
  transposes, freeing the tensor engine for pure matmul work. PV and the
  output projection consume the transposed tiles; normalization by the
  softmax rowsum is applied to O via a DRAM-roundtrip transposed-reciprocal
  broadcast. Output y.T [1024 cout, 2048 tok] fp16 partial.
"""

import sys

if "/opt/trn_rl_repo" not in sys.path:
    sys.path.insert(0, "/opt/trn_rl_repo")

from contextlib import ExitStack

import numpy as np

import concourse.bass as bass
import concourse.mybir as mybir
from concourse import bacc
from concourse.bass_utils import run_bass_kernel_spmd
from concourse.masks import make_identity
from concourse.tile import TileContext

F32 = mybir.dt.float32
BF16 = mybir.dt.bfloat16
FP16 = mybir.dt.float16
AF = mybir.ActivationFunctionType
ALU = mybir.AluOpType

DIM = 1024
HEADS = 8
HD = 128  # head dim
B, N = 4, 2048
SCALE = float(np.sqrt(DIM / HEADS))
NCORES = 8
TOK = 2048          # query tokens per core (whole batch)
KEYS = 2048         # keys per core (whole batch)
MARGIN = 66.0       # exp bias safety margin below sampled max


def _build():
    nc = bacc.Bacc("TRN2", target_bir_lowering=False, debug=False, num_devices=NCORES)

    # head-split sharding: each core owns HL=4 heads of one batch, all 2048
    # queries; the pair's partial output projections are summed on the host.
    xT_e = nc.declare_dram_parameter("xT", [2, 8, 128, 1024], FP16, isOutput=False)
    wqT_e = nc.declare_dram_parameter("wqT", [4, 8, 128, 128], FP16, isOutput=False)
    wkT_e = nc.declare_dram_parameter("wkT", [4, 8, 128, 128], FP16, isOutput=False)
    wvT_e = nc.declare_dram_parameter("wvT", [1, 8, 128, 512], FP16, isOutput=False)
    w0T_e = nc.declare_dram_parameter("w0T", [8, 4, 128, 128], FP16, isOutput=False)
    bq_e = nc.declare_dram_parameter("bq", [128, 4], F32, isOutput=False)
    b0_e = nc.declare_dram_parameter("b0", [128, 8], F32, isOutput=False)
    out_e = nc.declare_dram_parameter("out", [DIM, TOK], FP16, isOutput=True)
    HL = 4  # local heads per core

    with TileContext(nc) as tc, ExitStack() as ctx:
        persist = ctx.enter_context(tc.tile_pool(name="persist", bufs=1))
        QT = persist.tile([128, 4, TOK], FP16)         # [d, lhead, qtok]
        KT = persist.tile([128, 4, KEYS], FP16)        # [d, lhead, key]
        V = persist.tile([128, 16, 512], BF16)         # [tok%128, keytile, lfeat]
        bq_s = persist.tile([128, 4], F32)
        b0_s = persist.tile([128, 8], F32)

        # keep the gpsimd queue free for the x loads — small/persistent
        # inputs ride the sync queue, idle until the first XBAR
        nc.sync.dma_start(out=bq_s[:, :], in_=bq_e[:, :])
        nc.sync.dma_start(out=b0_s[:, :], in_=b0_e[:, :])

        # ---------------- QK projection, two token-half phases ----------------
        # V projection is deferred: its matmuls are interleaved into the
        # first two attention groups (the attention pipeline is ACT/XBAR-
        # bound, leaving tensor-engine slack). The x tiles stay resident in
        # SBUF for it — re-reading x from DRAM serializes the DMA queue.
        wvpool = ctx.enter_context(tc.tile_pool(name="wv", bufs=1))
        wv0 = wvpool.tile([128, 8, 512], FP16)
        nc.sync.dma_start(
            out=wv0[:, :, :], in_=wvT_e[0].rearrange("c p f -> p c f"))
        xpool = ctx.enter_context(tc.tile_pool(name="xT", bufs=1))
        xts = []
        with ExitStack() as qkv_ctx:
            wp128 = qkv_ctx.enter_context(tc.tile_pool(name="w128", bufs=4))
            pq = qkv_ctx.enter_context(tc.tile_pool(name="pq", bufs=6, space="PSUM"))

            for ph in range(2):
                xt = xpool.tile([128, 8, 1024], FP16, name=f"xt{ph}",
                                tag=f"xt{ph}")
                xts.append(xt)
                if ph == 0:
                    # land the first weight tile on queue 0 before the x chunks
                    wq0 = wp128.tile([128, 8, 128], FP16, tag="w128")
                    nc.gpsimd.dma_start(out=wq0[:, :, :],
                                        in_=wqT_e[0].rearrange("c p f -> p c f"))
                # split the x loads across two DMA queues to halve the fill
                # (the scalar queue is idle until the first exp at ~95us)
                for c in range(8):
                    eng = nc.gpsimd if c % 2 == 0 else nc.scalar
                    eng.dma_start(out=xt[:, c, :], in_=xT_e[ph, c])

                # Q projection for this half's queries
                for ft in range(4):
                    if ph == 0 and ft == 0:
                        wq = wq0
                    else:
                        wq = wp128.tile([128, 8, 128], FP16, tag="w128")
                        nc.gpsimd.dma_start(
                            out=wq[:, :, :],
                            in_=wqT_e[ft].rearrange("c p f -> p c f"))
                    for tch in range(2):
                        ps = pq.tile([128, 512], F32)
                        for c in range(8):
                            nc.tensor.matmul(
                                ps[:, :], wq[:, c, :],
                                xt[:, c, tch * 512:(tch + 1) * 512],
                                start=(c == 0), stop=(c == 7))
                        nc.scalar.activation(
                            QT[:, ft, ph * 1024 + tch * 512:
                               ph * 1024 + (tch + 1) * 512], ps[:, :],
                            AF.Identity, bias=bq_s[:, ft:ft + 1])

                # K projection for this half's keys (drain on DVE, off ACT)
                for ft in range(4):
                    wk = wp128.tile([128, 8, 128], FP16, tag="w128")
                    nc.gpsimd.dma_start(
                        out=wk[:, :, :],
                        in_=wkT_e[ft].rearrange("c p f -> p c f"))
                    for tch in range(2):
                        ps = pq.tile([128, 512], F32)
                        for c in range(8):
                            nc.tensor.matmul(
                                ps[:, :], wk[:, c, :],
                                xt[:, c, tch * 512:(tch + 1) * 512],
                                start=(c == 0), stop=(c == 7))
                        nc.vector.tensor_copy(
                            KT[:, ft, ph * 1024 + tch * 512:
                               ph * 1024 + (tch + 1) * 512],
                            ps[:, :])

        # ---------------- attention (+ deferred V proj, + output proj) ------
        wpool2 = ctx.enter_context(tc.tile_pool(name="w0", bufs=1))
        ypool = ctx.enter_context(tc.tile_pool(name="y", bufs=3))
        otpool = ctx.enter_context(tc.tile_pool(name="ot", bufs=1))
        # per-512-token-chunk output tiles so the output projection can start
        # as soon as a chunk's last head is normalized
        OT_t = [otpool.tile([128, 4, 512], FP16, name=f"ott{i}", tag=f"ott{i}")
                for i in range(4)]
        # prefetch all output-projection weights up front — on the sync
        # queue, which is idle until the first XBAR; putting these on the
        # gpsimd queue would delay the x loads behind them
        w0s = []
        for ct in range(8):
            w0 = wpool2.tile([128, 4, 128], FP16, tag=f"w0{ct}", name=f"w0{ct}")
            nc.sync.dma_start(
                out=w0[:, :, :], in_=w0T_e[ct].rearrange("c p f -> p c f"))
            w0s.append(w0)
        # opool/utpool live in the outer scope: the drain-time PV flushes
        # need them after the attention context (and its psum pools) close
        opool = ctx.enter_context(tc.tile_pool(name="ov", bufs=1, space="PSUM"))
        utpool = ctx.enter_context(tc.tile_pool(name="ut", bufs=2))
        with ExitStack() as att_ctx:
            spool = att_ctx.enter_context(tc.tile_pool(name="sc", bufs=3, space="PSUM"))
            aux = att_ctx.enter_context(tc.tile_pool(name="aux", bufs=1, space="PSUM"))
            upool = att_ctx.enter_context(tc.tile_pool(name="u", bufs=2))
            small = att_ctx.enter_context(tc.tile_pool(name="sm", bufs=16))

            # deferred V-projection jobs: two per query-tile slot in groups
            # 0-1 (all 16 must be EMITTED before group 2's first PV matmul
            # to avoid a same-engine ordering deadlock). The psum drain goes
            # on ACT, which has slack while the PE chews the extra matmuls.
            def v_job(ph, tt):
                def run():
                    ps = aux.tile([128, 512], F32, name="vps", tag="aux")
                    for c in range(8):
                        nc.tensor.matmul(
                            ps[:, :], xts[ph][:, c, tt * 128:(tt + 1) * 128],
                            wv0[:, c, :], start=(c == 0), stop=(c == 7))
                    nc.scalar.copy(V[:, ph * 8 + tt, :], ps[:, :])
                return run

            vjobs = [v_job(ph, tt) for ph in range(2) for tt in range(8)]
            # spread over groups 0-2 in key-tile order; group 2's jobs are
            # emitted BEFORE that slot's PV chunk, which needs exactly the
            # tile emitted there (chunk qi reads V tiles 4qi..4qi+3)
            vsched = {(0, 0): 2, (0, 1): 2, (0, 2): 1, (0, 3): 1,
                      (1, 0): 2, (1, 1): 2, (1, 2): 1, (1, 3): 1,
                      (2, 0): 1, (2, 1): 1, (2, 2): 1, (2, 3): 1}

            # software pipeline over the 16 (head, qgroup) groups: group g
            # emits its scores/exp, with the PV matmuls of group g-2
            # interleaved 4-per-qi so the tensor engine stays fed during the
            # ACT-bound exp phase and never waits on the XBAR DMA (which
            # completes during group g-1).
            pends = []  # (h, qg, UT, rc4)

            def pv_chunk(pend, ps_o, qi):
                h, qg, UT4 = pend
                for kt in range(qi * 4, qi * 4 + 4):
                    nc.tensor.matmul(
                        ps_o[:, :], V[:, kt, h * 128:(h + 1) * 128],
                        UT4[:, :, kt, :],
                        start=(kt == 0), stop=(kt == 15))

            def pv_tail(pend, ps_o):
                # probs are pre-normalized, so O drains with a plain copy
                h, qg, UT4 = pend
                nc.vector.tensor_copy(OT_t[qg][:, h, :], ps_o[:, :])

            def flush_pv(pend):
                ps_o = opool.tile([128, 512], F32, tag="ov", name="ps_o")
                for qi in range(4):
                    pv_chunk(pend, ps_o, qi)
                pv_tail(pend, ps_o)



            gi = 0
            for h in range(HL):
                for qg in range(4):
                    # [k%128, qi, keytile, q%128]: fully contiguous — the XBAR
                    # transpose requires a contiguous destination (strided
                    # dests produce wrong output on HW)
                    UT4 = utpool.tile([128, 4, 16, 128], BF16)
                    ug = upool.tile([128, 4, KEYS], BF16)
                    ready = pends.pop(0) if len(pends) >= 2 else None
                    ps_o = None
                    if ready is not None:
                        ps_o = opool.tile([128, 512], F32, tag="ov",
                                          name="ps_o")
                    for qi in range(4):
                        qt = qg * 4 + qi
                        q_sl = QT[:, h, qt * 128:(qt + 1) * 128]

                        negb = small.tile([128, 1], F32, tag="negb")
                        ra = small.tile([128, 1], F32, tag="ra")
                        rb = small.tile([128, 1], F32, tag="rb")
                        ps_halves = []
                        for half in range(2):
                            ps = spool.tile([128, 1024], F32, tag="sc")
                            ps_halves.append(ps)
                            for kc in range(2):
                                nc.tensor.matmul(
                                    ps[:, kc * 512:(kc + 1) * 512], q_sl,
                                    KT[:, h, half * 1024 + kc * 512:
                                       half * 1024 + (kc + 1) * 512],
                                    start=True, stop=True)
                        for _ in range(vsched.get((gi, qi), 0)):
                            if vjobs:
                                vjobs.pop(0)()
                        if ready is not None:
                            pv_chunk(ready, ps_o, qi)
                        mx = small.tile([128, 1], F32, tag="mx")
                        with tc.high_priority(offset=30):
                            nc.vector.tensor_reduce(
                                mx[:, :],
                                ps_halves[0][:, :].rearrange(
                                    "p (n s) -> p n s", s=4)[:, :, 0],
                                axis=mybir.AxisListType.X, op=ALU.max)
                        # on DVE right after the mx reduce — avoids a
                        # cross-engine hop on the scores->exp latency chain
                        nc.vector.tensor_scalar(
                            negb[:, :], mx[:, :], -1.0, -MARGIN,
                            op0=ALU.mult, op1=ALU.add)
                        for half in range(2):
                            nc.scalar.activation(
                                ug[:, qi, half * 1024:(half + 1) * 1024],
                                ps_halves[half][:, :],
                                AF.Exp, bias=negb[:, :], scale=1.0,
                                accum_out=(ra if half == 0 else rb)[:, :])

                        # normalize the probs in [q, k] layout, where the
                        # reciprocal rowsum is a per-partition scalar — PV
                        # output needs no further normalization
                        nc.vector.tensor_tensor(
                            out=ra[:, :], in0=ra[:, :], in1=rb[:, :],
                            op=ALU.add)
                        rc1 = small.tile([128, 1], F32, tag="rc1")
                        nc.vector.reciprocal(rc1[:, :], ra[:, :])
                        nc.vector.tensor_scalar(
                            ug[:, qi, :], ug[:, qi, :], rc1[:, :], None,
                            op0=ALU.mult)

                    # transpose the whole group's [4x 128 q, 2048 k] probs to
                    # [k, q] tiles with one DMA XBAR op on the sync HWDGE
                    # queue — no tensor-engine or DVE work. Keep it OFF the
                    # scalar queue: a waiting DMA at the ACT queue head
                    # stalls the exp stream behind it.
                    nc.sync.dma_start(
                        out=UT4[:, :, :, :], in_=ug[:, :, :], transpose=True)
                    gi += 1

                    if ready is not None:
                        pv_tail(ready, ps_o)
                    pends.append((h, qg, UT4))

        # attention psum pools (spool/aux) are closed here, freeing 7 banks
        # for the output projection while the final two PV flushes (which
        # only need opool/utpool, kept in the outer scope) still pend.

        # ---------------- output projection ----------------
        # tch-outer with per-chunk OT tiles: chunks 0/1 are already complete
        # ((h3,qg0)/(h3,qg1) flushed two groups back), so their projection
        # runs while the last two groups' XBARs and PV flushes finish.
        with ExitStack() as op_ctx:
            pyp = op_ctx.enter_context(tc.tile_pool(name="py", bufs=5, space="PSUM"))

            def outproj_tch(tch):
                for ct in range(8):
                    ps = pyp.tile([128, 512], F32, name="yps")
                    for dc in range(4):
                        nc.tensor.matmul(
                            ps[:, :], w0s[ct][:, dc, :],
                            OT_t[tch][:, dc, :],
                            start=(dc == 0), stop=(dc == 3))
                    y = ypool.tile([128, 512], FP16, name="y")
                    nc.scalar.activation(
                        y[:, :], ps[:, :], AF.Identity,
                        bias=b0_s[:, ct:ct + 1])
                    nc.sync.dma_start(
                        out=out_e[ct * 128:(ct + 1) * 128,
                                  tch * 512:(tch + 1) * 512],
                        in_=y[:, :])

            outproj_tch(0)
            outproj_tch(1)
            flush_pv(pends.pop(0))
            outproj_tch(2)
            flush_pv(pends.pop(0))
            outproj_tch(3)

    nc.compile()
    return nc


_NC = None


def _get_nc():
    global _NC
    if _NC is None:
        _NC = _build()
    return _NC


def _make_in_maps(x, W_qkv, b_qkv, W0, b0):
    x = np.asarray(x, dtype=np.float32)
    W_qkv = np.asarray(W_qkv, dtype=np.float32)
    b_qkv = np.asarray(b_qkv, dtype=np.float32)
    W0 = np.asarray(W0, dtype=np.float32)
    b0 = np.asarray(b0, dtype=np.float32)

    def tile_w(wT, fsz):
        # [1024 cin, F] -> [F/fsz, 8, 128, fsz] contiguous
        nf = wT.shape[1] // fsz
        return np.ascontiguousarray(
            wT.reshape(8, 128, nf, fsz).transpose(2, 0, 1, 3)
        ).astype(np.float16)

    # V-bias folds through the output projection (softmax rows sum to 1);
    # K-bias only shifts each score row uniformly, which softmax cancels.
    # Each core of a pair adds half of the effective output bias.
    b0_eff = 0.5 * (b0 + W0 @ b_qkv[2 * DIM:3 * DIM])
    b0r = np.ascontiguousarray(b0_eff.reshape(8, 128).T).astype(np.float32)

    in_maps = []
    for c in range(NCORES):
        b, g = c // 2, c % 2
        hs = slice(g * 512, (g + 1) * 512)  # this core's 4 heads (features)
        wqT = tile_w((W_qkv[0:DIM] * SCALE).T[:, hs], 128)
        wkT = tile_w(W_qkv[DIM:2 * DIM].T[:, hs], 128)
        wvT = tile_w(W_qkv[2 * DIM:3 * DIM].T[:, hs], 512)
        # w0T rows for this head group: [512 din, 1024 cout] -> [8ct, 4c, 128, 128]
        w0T = np.ascontiguousarray(
            W0.T[g * 512:(g + 1) * 512].reshape(4, 128, 8, 128)
            .transpose(2, 0, 1, 3)).astype(np.float16)
        bq = np.ascontiguousarray(
            (b_qkv[0:DIM] * SCALE)[hs].reshape(4, 128).T).astype(np.float32)
        xT = np.ascontiguousarray(
            x[b].T.reshape(8, 128, 2, 1024).transpose(2, 0, 1, 3)
        ).astype(np.float16)
        in_maps.append({
            "xT": xT, "wqT": wqT, "wkT": wkT, "wvT": wvT, "w0T": w0T,
            "bq": bq, "b0": b0r,
        })
    return in_maps


def _assemble(results):
    y = np.empty((B, N, DIM), dtype=np.float32)
    for b in range(B):
        y[b] = (results[2 * b]["out"].astype(np.float32)
                + results[2 * b + 1]["out"].astype(np.float32)).T
    return y


def kernel(x, W_qkv, b_qkv, W0, b0):
    nc = _get_nc()
    in_maps = _make_in_maps(x, W_qkv, b_qkv, W0, b0)
    res = run_bass_kernel_spmd(nc, in_maps, core_ids=list(range(NCORES)))
    return _assemble(res.results)


def kernel_traced(x, W_qkv, b_qkv, W0, b0, tmpdir=None):
    """Same as kernel() but with NTFF profiling; returns (output, BassKernelResults)."""
    nc = _get_nc()
    in_maps = _make_in_maps(x, W_qkv, b_qkv, W0, b0)
    res = run_bass_kernel_spmd(nc, in_maps, core_ids=list(range(NCORES)),
                               trace=True, trace_cores=[0], tmpdir=tmpdir)
    return _assemble(res.results), res
